# revision 19
# baseline (speedup 1.0000x reference)
"""EquivariantGNN message-passing kernel for Trainium2 (8 NeuronCores, SPMD).

Math (matches the reference):
  x   = [pos | onehot(z)] @ [[I3,0],[0,emb]]          (rank-8 node features)
  q/k/v = x @ W* = x8 @ W*8        with W*8 = [[W*[:3]],[emb @ W*[3:]]]  (8x128)
  ke  = k[src] + ea@We = [ea | x8[src]] @ [[We],[Wk8]]
  logits[e,h] = 0.25 * q[dst]. ke[e]  =  sum_i x8[dst][i] * G[e, h*8+i]
      where G = [ea | x8[src]] @ Bcat   (Bcat[j, h*8+i] = 0.25 * Wq8[i,hd].Wke12[j,hd])
  w = exp(logits)  (no max subtraction needed; logits are O(10))
  den[n,h] = sum_{dst(e)=n} w ;  agg[n] = (sum w*ve) / (den+1e-9)
  out = agg @ Wo + x ; S = sum_n relu(out) ; answer = (S @ lin_w)/N + lin_b

Device strategy per core: edges sorted by dst, 128-edge blocks each fully inside
one 128-node tile.  Per block: indirect-DMA gather of x8[src] (32B rows) into a
[128,12] tile (with ea), PE transpose -> lhsT12, one f32r matmul -> ke|ve|G,
DVE logits + exp, onehot(localdst) matmul scatter-accumulate of [w|w*ve] into a
per-tile PSUM accumulator.  Tile epilogue normalizes, applies Wo + residual,
relu, and accumulates the node-sum S via a ones-matmul.
"""

import math
import os
import sys

import numpy as np

for _p in ("/opt/trn_rl_repo", "/root/.axon_site/_ro/trn_rl_repo"):
    if os.path.isdir(_p) and _p not in sys.path:
        sys.path.insert(0, _p)

P = 128
DIM = 128
H, DH = 8, 16
DE = 4
N_CORES = 8
GCHUNK = 64  # blocks per gather chunk

# test-harness knobs (the grading harness just calls kernel() with defaults)
PROFILE = False
TRACE_CORES = None
LAST_RESULT = None  # BassKernelResults of the last run (for profiling)
_PROG_CACHE = {}


# ---------------------------------------------------------------- host prep
def _host_prep(pos, edge_attr, emb, Wq, Wk, Wv, We, z, edge_index):
    f32 = np.float32
    N = pos.shape[0]
    NT = emb.shape[0]
    ntiles = (N + P - 1) // P
    npad = ntiles * P

    z = np.asarray(z).astype(np.int64)
    src = np.asarray(edge_index[0]).astype(np.int64)
    dst = np.asarray(edge_index[1]).astype(np.int64)
    E = src.shape[0]

    onehot = np.zeros((N, NT), f32)
    onehot[np.arange(N), z] = 1.0
    x8 = np.concatenate([np.asarray(pos, f32), onehot], axis=1)  # [N, 8]
    x8p = np.zeros((npad, 8), f32)
    x8p[:N] = x8

    # rank-8 weight factors
    Wq8 = np.vstack([Wq[:3], emb @ Wq[3:]]).astype(f32)  # [8,128]
    Wk8 = np.vstack([Wk[:3], emb @ Wk[3:]]).astype(f32)
    Wv8 = np.vstack([Wv[:3], emb @ Wv[3:]]).astype(f32)
    Wke12 = np.vstack([Wk8, We]).astype(f32)  # [12,128]: rows = [x8src(8); ea(4)]

    # bilinear logits factors: logits[e,h] = sum_i x8dst[i] * G[e,h*8+i],
    # G[e, h*8+i] = sum_j lhs12[j,e] * Bc[j, h*8+i],  lhs12 rows = [ea(4); x8src(8)]
    Bc = np.zeros((12, 64), f32)
    for h in range(H):
        Bh = Wq8[:, h * DH:(h + 1) * DH] @ Wke12[:, h * DH:(h + 1) * DH].T  # [8,12]
        Bc[0:4, h * 8:(h + 1) * 8] = 0.25 * Bh[:, 8:12].T  # ea rows
        Bc[4:12, h * 8:(h + 1) * 8] = 0.25 * Bh[:, 0:8].T  # src rows
    rhs12 = np.zeros((12, 320), f32)
    rhs12[0:4, 0:128] = We
    rhs12[0:4, 128:256] = We
    rhs12[4:12, 0:128] = Wk8
    rhs12[4:12, 128:256] = Wv8
    rhs12[:, 256:320] = Bc

    J8 = np.zeros((8, DIM), f32)  # x = x8 @ J8
    J8[0:3, 0:3] = np.eye(3, dtype=f32)
    J8[3:8, 3:DIM] = emb

    # ---- sort edges by dst, split into per-node-tile runs
    perm = np.argsort(dst, kind="stable")
    src_s, dst_s = src[perm], dst[perm]
    ea_s = np.asarray(edge_attr, f32)[perm]
    tile_of_edge = dst_s // P
    starts = np.searchsorted(tile_of_edge, np.arange(ntiles))
    ends = np.searchsorted(tile_of_edge, np.arange(ntiles) + 1)
    ecnt = ends - starts
    nb = np.maximum(1, (ecnt + P - 1) // P)  # blocks per real tile

    # ---- uniform schedule across cores: pad tile list to multiple of 8,
    # sort by block count desc, deal groups of 8 (one tile per core),
    # pad each group to the group max -> identical counts on every core.
    ntiles_tot = ((ntiles + N_CORES - 1) // N_CORES) * N_CORES
    nb_all = np.concatenate([nb, np.ones(ntiles_tot - ntiles, np.int64)])
    order = np.argsort(-nb_all, kind="stable")
    TS = ntiles_tot // N_CORES  # tiles per core
    counts = [int(nb_all[order[8 * k]]) for k in range(TS)]  # group max (sorted desc)
    C = int(sum(counts))

    srcfac = np.zeros((N_CORES, C, P, 12), f32)  # [ea(4) | x8[src](8)]
    dstfac = np.zeros((N_CORES, C, P, 8), f32)  # x8[dst]
    ldst = np.full((N_CORES, C, P), 255.0, f32)
    xT8c = np.zeros((N_CORES, 8, TS * P), f32)

    offs = np.concatenate([[0], np.cumsum(counts)])
    for k in range(TS):
        for j in range(N_CORES):
            t = int(order[8 * k + j])
            if t >= ntiles:
                continue  # dummy tile: all-dummy blocks, zero xT8c
            xT8c[j, :, k * P:(k + 1) * P] = x8p[t * P:(t + 1) * P].T
            e0, e1 = int(starts[t]), int(ends[t])
            ne = e1 - e0
            if ne == 0:
                continue
            c0 = int(offs[k])
            flat = np.arange(ne)
            cc = c0 + flat // P
            pp = flat % P
            srcfac[j, cc, pp, 0:DE] = ea_s[e0:e1]
            srcfac[j, cc, pp, DE:12] = x8[src_s[e0:e1]]
            dstfac[j, cc, pp, :] = x8[dst_s[e0:e1]]
            ldst[j, cc, pp] = (dst_s[e0:e1] - t * P).astype(f32)

    iota = np.tile(np.arange(P, dtype=f32), (P, 1))
    ident = np.eye(P, dtype=f32)
    ones = np.ones((P, 1), f32)

    shared = dict(rhs12=rhs12, J8=J8, iota=iota, ident=ident, ones=ones)
    percore = dict(srcfac=srcfac, dstfac=dstfac, ldst=ldst, xT8c=xT8c)
    meta = dict(counts=counts, C=C, TS=TS, npad=npad, N=N, E=E)
    return shared, percore, meta


# ---------------------------------------------------------------- device code
DBG_T = 0  # tile index to tap when dbg=True
DBG_G = 0  # global block index to tap when dbg=True


def _build_program(counts, C, TS, npad, use_f32r=True, scatter_bf16=True, dbg=False):
    import concourse.bacc as bacc
    import concourse.bass as bass
    import concourse.tile as tile
    from concourse import mybir
    from concourse._compat import with_exitstack  # noqa: F401

    f32 = mybir.dt.float32
    f32r = mybir.dt.float32r if use_f32r else mybir.dt.float32
    bf16 = mybir.dt.bfloat16
    i32 = mybir.dt.int32
    sdt = bf16 if scatter_bf16 else f32

    nc = bacc.Bacc("TRN2", target_bir_lowering=False, debug=False,
                   enable_asserts=False, num_devices=N_CORES)

    srcfac_in = nc.dram_tensor("srcfac", [P, C, 12], f32, kind="ExternalInput").ap()
    dstfac_in = nc.dram_tensor("dstfac", [P, C, 8], f32, kind="ExternalInput").ap()
    ldst_in = nc.dram_tensor("ldst", [P, C], f32, kind="ExternalInput").ap()
    xT8c_in = nc.dram_tensor("xT8c", [8, TS * P], f32r, kind="ExternalInput").ap()
    rhs12_in = nc.dram_tensor("rhs12", [12, 320], f32r, kind="ExternalInput").ap()
    J8_in = nc.dram_tensor("J8", [8, DIM], f32r, kind="ExternalInput").ap()
    Wo_in = nc.dram_tensor("Wo_in", [DIM, DIM], f32r, kind="ExternalInput").ap()
    iota_in = nc.dram_tensor("iota", [P, P], f32, kind="ExternalInput").ap()
    ident_in = nc.dram_tensor("ident", [P, P], f32, kind="ExternalInput").ap()
    ones_in = nc.dram_tensor("ones", [P, 1], f32, kind="ExternalInput").ap()
    S_out = nc.dram_tensor("S_out", [1, DIM], f32, kind="ExternalOutput").ap()
    dbg_outs = {}
    if dbg:
        for nm, shp in [("d_src12", [P, 12]), ("d_dst8", [P, 8]),
                        ("d_lhsT", [12, P]), ("d_psm", [P, 320]),
                        ("d_gx", [P, 64]), ("d_lg", [P, H]),
                        ("d_rhswm", [P, 8 + DIM]), ("d_oh", [P, P]),
                        ("d_acc", [P, 8 + DIM]), ("d_aggs", [P, DIM]),
                        ("d_pso", [P, DIM]), ("d_hrelu", [P, DIM])]:
            dbg_outs[nm] = nc.dram_tensor(nm, shp, f32, kind="ExternalOutput").ap()

    with tile.TileContext(nc) as tc:
        with (
            tc.tile_pool(name="const", bufs=1) as constp,
            tc.tile_pool(name="blk", bufs=3) as blkp,
            tc.tile_pool(name="psmain", bufs=2, space="PSUM") as psmainp,
            tc.tile_pool(name="psmisc", bufs=2, space="PSUM") as psmiscp,
            tc.tile_pool(name="psacc", bufs=2, space="PSUM") as psaccp,
            tc.tile_pool(name="psS", bufs=1, space="PSUM") as psSp,
        ):
            sc = constp.tile_from(srcfac_in)
            dc = constp.tile_from(dstfac_in)
            ldst_sb = constp.tile_from(ldst_in)
            xT8c_sb = constp.tile_from(xT8c_in)
            rhs12_sb = constp.tile_from(rhs12_in)
            J8_sb = constp.tile_from(J8_in)
            Wo_sb = constp.tile_from(Wo_in)
            iota_sb = constp.tile_from(iota_in)
            ident_sb = constp.tile_from(ident_in)
            ones_sb = constp.tile_from(ones_in)

            psS = psSp.tile([1, DIM], f32, tag="S")

            def tap(name, ap):
                if not dbg or name not in dbg_outs:
                    return
                tmp = constp.tile(list(ap.shape), f32, tag="tap_" + name)
                nc.vector.tensor_copy(tmp[:], ap)
                nc.sync.dma_start(out=dbg_outs[name], in_=tmp[:])

            g = 0
            for t in range(TS):
                nb = counts[t]
                acc = psaccp.tile([P, 8 + DIM], f32, tag="acc")
                for b in range(nb):
                    psT = psmiscp.tile([12, P], f32, tag="T")
                    nc.tensor.transpose(out=psT[:], in_=sc[:, g, :], identity=ident_sb[:])
                    lhsT12 = blkp.tile([12, P], f32r, tag="lhsT")
                    nc.scalar.copy(lhsT12[:], psT[:])

                    psm = psmainp.tile([P, 320], f32, tag="main")
                    nc.tensor.matmul(psm[:], lhsT=lhsT12[:], rhs=rhs12_sb[:],
                                     start=True, stop=True)

                    gx = blkp.tile([P, 64], f32, tag="gx")
                    nc.vector.tensor_tensor(
                        out=gx[:].rearrange("p (a b) -> p a b", b=8),
                        in0=psm[:, 256:320].rearrange("p (a b) -> p a b", b=8),
                        in1=dc[:, g, None, :].to_broadcast([P, 8, 8]),
                        op=mybir.AluOpType.mult,
                    )
                    lg = blkp.tile([P, H], f32, tag="lg")
                    nc.vector.tensor_reduce(
                        out=lg[:], in_=gx[:].rearrange("p (a b) -> p a b", b=8),
                        axis=mybir.AxisListType.X, op=mybir.AluOpType.add,
                    )
                    w = blkp.tile([P, H], f32, tag="w")
                    nc.scalar.activation(w[:], lg[:], mybir.ActivationFunctionType.Exp)
                    rhswm = blkp.tile([P, 8 + DIM], sdt, tag="rhswm")
                    nc.scalar.activation(rhswm[:, 0:8], lg[:], mybir.ActivationFunctionType.Exp)
                    nc.vector.tensor_tensor(
                        out=rhswm[:, 8:8 + DIM].rearrange("p (a b) -> p a b", b=DH),
                        in0=psm[:, 128:256].rearrange("p (a b) -> p a b", b=DH),
                        in1=w[:, :, None].to_broadcast([P, H, DH]),
                        op=mybir.AluOpType.mult,
                    )
                    oh = blkp.tile([P, P], sdt, tag="oh")
                    nc.gpsimd.tensor_scalar(
                        out=oh[:], in0=iota_sb[:], scalar1=ldst_sb[:, g:g + 1],
                        scalar2=None, op0=mybir.AluOpType.is_equal,
                    )
                    nc.tensor.matmul(acc[:], lhsT=oh[:], rhs=rhswm[:],
                                     start=(b == 0), stop=(b == nb - 1))
                    if dbg and g == DBG_G:
                        tap("d_src12", sc[:, g, :])
                        tap("d_dst8", dc[:, g, :])
                        tap("d_lhsT", lhsT12[:])
                        tap("d_psm", psm[:])
                        tap("d_gx", gx[:])
                        tap("d_lg", lg[:])
                        tap("d_rhswm", rhswm[:])
                        tap("d_oh", oh[:])
                    g += 1

                # -------- tile epilogue
                den = blkp.tile([P, H], f32, tag="den")
                nc.vector.tensor_scalar_add(den[:], acc[:, 0:8], 1e-9)
                rden = blkp.tile([P, H], f32, tag="rden")
                nc.vector.reciprocal(rden[:], den[:])
                aggs = blkp.tile([P, DIM], f32, tag="aggs")
                nc.vector.tensor_tensor(
                    out=aggs[:].rearrange("p (a b) -> p a b", b=DH),
                    in0=acc[:, 8:8 + DIM].rearrange("p (a b) -> p a b", b=DH),
                    in1=rden[:, :, None].to_broadcast([P, H, DH]),
                    op=mybir.AluOpType.mult,
                )
                psT2 = psmiscp.tile([P, P], f32, tag="T")
                nc.tensor.transpose(out=psT2[:], in_=aggs[:], identity=ident_sb[:])
                aggT = blkp.tile([P, P], f32r, tag="aggT")
                nc.scalar.copy(aggT[:], psT2[:])
                pso = psmiscp.tile([P, DIM], f32, tag="T")
                nc.tensor.matmul(pso[:], lhsT=aggT[:], rhs=Wo_sb[:],
                                 start=True, stop=False)
                nc.tensor.matmul(pso[:], lhsT=xT8c_sb[:, t * P:(t + 1) * P],
                                 rhs=J8_sb[:], start=False, stop=True)
                hrelu = blkp.tile([P, DIM], f32, tag="hrelu")
                nc.scalar.activation(hrelu[:], pso[:], mybir.ActivationFunctionType.Relu)
                nc.tensor.matmul(psS[:], lhsT=ones_sb[:], rhs=hrelu[:],
                                 start=(t == 0), stop=(t == TS - 1))
                if dbg and t == DBG_T:
                    tap("d_acc", acc[:])
                    tap("d_aggs", aggs[:])
                    tap("d_pso", pso[:])
                    tap("d_hrelu", hrelu[:])

            Scopy = constp.tile([1, DIM], f32, tag="Scopy")
            nc.vector.tensor_copy(Scopy[:], psS[:])
            nc.sync.dma_start(out=S_out, in_=Scopy[:])

    nc.compile()
    return nc


# ---------------------------------------------------------------- entry point
def kernel(**inputs):
    pos = np.asarray(inputs["pos"], np.float32)
    edge_attr = np.asarray(inputs["edge_attr"], np.float32)
    emb = np.asarray(inputs["emb"], np.float32)
    Wq = np.asarray(inputs["Wq"], np.float32)
    Wk = np.asarray(inputs["Wk"], np.float32)
    Wv = np.asarray(inputs["Wv"], np.float32)
    We = np.asarray(inputs["We"], np.float32)
    Wo = np.asarray(inputs["Wo"], np.float32)
    lin_w = np.asarray(inputs["lin_w"], np.float32)
    lin_b = np.asarray(inputs["lin_b"], np.float32)
    z = inputs["z"]
    edge_index = inputs["edge_index"]

    shared, percore, meta = _host_prep(pos, edge_attr, emb, Wq, Wk, Wv, We, z, edge_index)
    N = meta["N"]

    key = (tuple(meta["counts"]), meta["C"], meta["TS"], meta["npad"])
    nc = _PROG_CACHE.get(key)
    if nc is None:
        nc = _build_program(meta["counts"], meta["C"], meta["TS"], meta["npad"])
        _PROG_CACHE[key] = nc

    in_maps = []
    for j in range(N_CORES):
        m = {
            "rhs12": shared["rhs12"],
            "J8": shared["J8"],
            "iota": shared["iota"],
            "ident": shared["ident"],
            "ones": shared["ones"],
            "Wo_in": Wo,
            "srcfac": np.ascontiguousarray(percore["srcfac"][j].transpose(1, 0, 2)),
            "dstfac": np.ascontiguousarray(percore["dstfac"][j].transpose(1, 0, 2)),
            "ldst": np.ascontiguousarray(percore["ldst"][j].T),
            "xT8c": percore["xT8c"][j],
        }
        in_maps.append(m)

    from concourse.bass_utils import run_bass_kernel_spmd
    res = run_bass_kernel_spmd(nc, in_maps, core_ids=list(range(N_CORES)),
                               trace=PROFILE, trace_cores=TRACE_CORES)
    global LAST_RESULT
    LAST_RESULT = res
    S = np.zeros(DIM, np.float64)
    for r in res.results:
        S += r["S_out"][0].astype(np.float64)
    y = (S.astype(np.float32) @ lin_w) / np.float32(N) + lin_b
    return y.reshape(1, 1).astype(np.float32)


# revision 26
# speedup vs baseline: 2.0389x; 2.0389x over previous
"""EquivariantGNN message-passing kernel for Trainium2 (8 NeuronCores, SPMD).

Math (matches the reference):
  x   = [pos | onehot(z)] @ [[I3,0],[0,emb]]          (rank-8 node features)
  q/k/v = x @ W* = x8 @ W*8        with W*8 = [[W*[:3]],[emb @ W*[3:]]]  (8x128)
  ke  = k[src] + ea@We = [ea | x8[src]] @ [[We],[Wk8]]
  logits[e,h] = 0.25 * q[dst]. ke[e]  =  sum_i x8[dst][i] * G[e, h*8+i]
      where G = [ea | x8[src]] @ Bcat   (Bcat[j, h*8+i] = 0.25 * Wq8[i,hd].Wke12[j,hd])
  w = exp(logits)  (no max subtraction needed; logits are O(10))
  den[n,h] = sum_{dst(e)=n} w ;  agg[n] = (sum w*ve) / (den+1e-9)
  out = agg @ Wo + x ; S = sum_n relu(out) ; answer = (S @ lin_w)/N + lin_b

Device strategy per core: edges sorted by dst, 128-edge blocks each fully inside
one 128-node tile.  Per block: indirect-DMA gather of x8[src] (32B rows) into a
[128,12] tile (with ea), PE transpose -> lhsT12, one f32r matmul -> ke|ve|G,
DVE logits + exp, onehot(localdst) matmul scatter-accumulate of [w|w*ve] into a
per-tile PSUM accumulator.  Tile epilogue normalizes, applies Wo + residual,
relu, and accumulates the node-sum S via a ones-matmul.
"""

import math
import os
import sys

import numpy as np

for _p in ("/opt/trn_rl_repo", "/root/.axon_site/_ro/trn_rl_repo"):
    if os.path.isdir(_p) and _p not in sys.path:
        sys.path.insert(0, _p)

P = 128
DIM = 128
H, DH = 8, 16
DE = 4
N_CORES = 8
GCHUNK = 64  # blocks per gather chunk

# test-harness knobs (the grading harness just calls kernel() with defaults)
PROFILE = False
TRACE_CORES = None
LAST_RESULT = None  # BassKernelResults of the last run (for profiling)
_PROG_CACHE = {}


# ---------------------------------------------------------------- host prep
def _host_prep(pos, edge_attr, emb, Wq, Wk, Wv, We, z, edge_index):
    f32 = np.float32
    N = pos.shape[0]
    NT = emb.shape[0]
    ntiles = (N + P - 1) // P
    npad = ntiles * P

    z = np.asarray(z).astype(np.int64)
    src = np.asarray(edge_index[0]).astype(np.int64)
    dst = np.asarray(edge_index[1]).astype(np.int64)
    E = src.shape[0]

    onehot = np.zeros((N, NT), f32)
    onehot[np.arange(N), z] = 1.0
    x8 = np.concatenate([np.asarray(pos, f32), onehot], axis=1)  # [N, 8]
    x8p = np.zeros((npad, 8), f32)
    x8p[:N] = x8

    # rank-8 weight factors
    Wq8 = np.vstack([Wq[:3], emb @ Wq[3:]]).astype(f32)  # [8,128]
    Wk8 = np.vstack([Wk[:3], emb @ Wk[3:]]).astype(f32)
    Wv8 = np.vstack([Wv[:3], emb @ Wv[3:]]).astype(f32)
    Wke12 = np.vstack([Wk8, We]).astype(f32)  # [12,128]: rows = [x8src(8); ea(4)]

    # bilinear logits factors: logits[e,h] = sum_i x8dst[i] * G[e,h*8+i],
    # G[e, h*8+i] = sum_j lhs12[j,e] * Bc[j, h*8+i],  lhs12 rows = [ea(4); x8src(8)]
    Bc = np.zeros((12, 64), f32)
    for h in range(H):
        Bh = Wq8[:, h * DH:(h + 1) * DH] @ Wke12[:, h * DH:(h + 1) * DH].T  # [8,12]
        Bc[0:4, h * 8:(h + 1) * 8] = 0.25 * Bh[:, 8:12].T  # ea rows
        Bc[4:12, h * 8:(h + 1) * 8] = 0.25 * Bh[:, 0:8].T  # src rows
    rhs12 = np.zeros((12, 320), f32)
    rhs12[0:4, 0:128] = We
    rhs12[0:4, 128:256] = We
    rhs12[4:12, 0:128] = Wk8
    rhs12[4:12, 128:256] = Wv8
    rhs12[:, 256:320] = Bc

    J8 = np.zeros((8, DIM), f32)  # x = x8 @ J8
    J8[0:3, 0:3] = np.eye(3, dtype=f32)
    J8[3:8, 3:DIM] = emb

    # ---- sort edges by dst, split into per-node-tile runs
    perm = np.argsort(dst, kind="stable")
    src_s, dst_s = src[perm], dst[perm]
    ea_s = np.asarray(edge_attr, f32)[perm]
    tile_of_edge = dst_s // P
    starts = np.searchsorted(tile_of_edge, np.arange(ntiles))
    ends = np.searchsorted(tile_of_edge, np.arange(ntiles) + 1)
    ecnt = ends - starts
    nb = np.maximum(1, (ecnt + P - 1) // P)  # blocks per real tile

    # ---- uniform schedule across cores: pad tile list to multiple of 8,
    # sort by block count desc, deal groups of 8 (one tile per core),
    # pad each group to the group max -> identical counts on every core.
    ntiles_tot = ((ntiles + N_CORES - 1) // N_CORES) * N_CORES
    nb_all = np.concatenate([nb, np.ones(ntiles_tot - ntiles, np.int64)])
    order = np.argsort(-nb_all, kind="stable")
    TS = ntiles_tot // N_CORES  # tiles per core
    counts = [int(nb_all[order[8 * k]]) for k in range(TS)]  # group max (sorted desc)
    if sum(counts) % 2:
        counts[-1] += 1  # keep global block count even for pair processing
    C = int(sum(counts))

    import ml_dtypes

    srcfac = np.zeros((N_CORES, C, P, 12), f32)  # [ea(4) | x8[src](8)]
    dstfac = np.zeros((N_CORES, C, P, 8), f32)  # x8[dst]
    ohmat = np.zeros((N_CORES, C, P, P), ml_dtypes.bfloat16)  # onehot(localdst)
    xT8c = np.zeros((N_CORES, 8, TS * P), f32)

    offs = np.concatenate([[0], np.cumsum(counts)])
    for k in range(TS):
        for j in range(N_CORES):
            t = int(order[8 * k + j])
            if t >= ntiles:
                continue  # dummy tile: all-dummy blocks, zero xT8c
            xT8c[j, :, k * P:(k + 1) * P] = x8p[t * P:(t + 1) * P].T
            e0, e1 = int(starts[t]), int(ends[t])
            ne = e1 - e0
            if ne == 0:
                continue
            c0 = int(offs[k])
            flat = np.arange(ne)
            cc = c0 + flat // P
            pp = flat % P
            srcfac[j, cc, pp, 0:DE] = ea_s[e0:e1]
            srcfac[j, cc, pp, DE:12] = x8[src_s[e0:e1]]
            dstfac[j, cc, pp, :] = x8[dst_s[e0:e1]]
            ohmat[j, cc, pp, dst_s[e0:e1] - t * P] = 1.0

    ident = np.eye(P, dtype=f32)
    ones = np.ones((P, 1), f32)

    # device layouts
    srcfacT = np.ascontiguousarray(
        srcfac.transpose(0, 3, 1, 2)).reshape(N_CORES, 12, C * P)
    dstfacd = np.ascontiguousarray(dstfac.transpose(0, 2, 1, 3))  # [j, P, C, 8]
    ohmatd = np.ascontiguousarray(ohmat.transpose(0, 2, 1, 3))  # [j, P, C, P]

    shared = dict(rhs12=rhs12, J8=J8, ident=ident, ones=ones)
    percore = dict(srcfacT=srcfacT, dstfac=dstfacd, ohmat=ohmatd, xT8c=xT8c)
    meta = dict(counts=counts, C=C, TS=TS, npad=npad, N=N, E=E)
    return shared, percore, meta


# ---------------------------------------------------------------- device code
DBG_T = 0  # tile index to tap when dbg=True
DBG_G = 0  # global block index to tap when dbg=True


def _build_program(counts, C, TS, npad, use_f32r=True, scatter_bf16=True, dbg=False):
    import concourse.bacc as bacc
    import concourse.bass as bass
    import concourse.tile as tile
    from concourse import mybir
    from concourse._compat import with_exitstack  # noqa: F401

    f32 = mybir.dt.float32
    f32r = mybir.dt.float32r if use_f32r else mybir.dt.float32
    bf16 = mybir.dt.bfloat16
    i32 = mybir.dt.int32
    sdt = bf16 if scatter_bf16 else f32

    nc = bacc.Bacc("TRN2", target_bir_lowering=False, debug=False,
                   enable_asserts=False, num_devices=N_CORES)

    srcfacT_in = nc.dram_tensor("srcfacT", [12, C * P], f32r, kind="ExternalInput").ap()
    dstfac_in = nc.dram_tensor("dstfac", [P, C, 8], f32, kind="ExternalInput").ap()
    ohmat_in = nc.dram_tensor("ohmat", [P, C, P], sdt, kind="ExternalInput").ap()
    xT8c_in = nc.dram_tensor("xT8c", [8, TS * P], f32r, kind="ExternalInput").ap()
    rhs12_in = nc.dram_tensor("rhs12", [12, 320], f32r, kind="ExternalInput").ap()
    J8_in = nc.dram_tensor("J8", [8, DIM], f32r, kind="ExternalInput").ap()
    Wo_in = nc.dram_tensor("Wo_in", [DIM, DIM], f32r, kind="ExternalInput").ap()
    ident_in = nc.dram_tensor("ident", [P, P], f32, kind="ExternalInput").ap()
    ones_in = nc.dram_tensor("ones", [P, 1], f32, kind="ExternalInput").ap()
    S_out = nc.dram_tensor("S_out", [1, DIM], f32, kind="ExternalOutput").ap()
    dbg_outs = {}
    if dbg:
        for nm, shp in [("d_dst8", [P, 8]), ("d_psm", [P, 320]),
                        ("d_gx", [P, 64]), ("d_lg", [P, H]),
                        ("d_rhswm", [P, 8 + DIM]), ("d_oh", [P, P]),
                        ("d_acc", [P, 8 + DIM]), ("d_aggs", [P, DIM]),
                        ("d_pso", [P, DIM]), ("d_hrelu", [P, DIM])]:
            dbg_outs[nm] = nc.dram_tensor(nm, shp, f32, kind="ExternalOutput").ap()

    with tile.TileContext(nc) as tc:
        with (
            tc.tile_pool(name="const", bufs=1) as constp,
            tc.tile_pool(name="chunks", bufs=2) as chunkp,
            tc.tile_pool(name="blk", bufs=3) as blkp,
            tc.tile_pool(name="psmain", bufs=2, space="PSUM") as psmainp,
            tc.tile_pool(name="psmisc", bufs=1, space="PSUM") as psmiscp,
            tc.tile_pool(name="psacc", bufs=2, space="PSUM") as psaccp,
            tc.tile_pool(name="psS", bufs=1, space="PSUM") as psSp,
        ):
            dc = constp.tile_from(dstfac_in)
            xT8c_sb = constp.tile_from(xT8c_in)
            rhs12_sb = constp.tile_from(rhs12_in)
            J8_sb = constp.tile_from(J8_in)
            Wo_sb = constp.tile_from(Wo_in)
            ident_sb = constp.tile_from(ident_in)
            ones_sb = constp.tile_from(ones_in)

            psS = psSp.tile([1, DIM], f32, tag="S")

            def tap(name, ap):
                if not dbg or name not in dbg_outs:
                    return
                tmp = constp.tile(list(ap.shape), f32, tag="tap_" + name)
                nc.vector.tensor_copy(tmp[:], ap)
                nc.sync.dma_start(out=dbg_outs[name], in_=tmp[:])

            # block -> (tile, b, nb) map for the flat pair loop
            blk2tile = []
            for t in range(TS):
                for b in range(counts[t]):
                    blk2tile.append((t, b, counts[t]))

            def _epilogue(t, acc):
                den = blkp.tile([P, H], f32, tag="den")
                nc.vector.tensor_scalar_add(den[:], acc[:, 0:8], 1e-9)
                rden = blkp.tile([P, H], f32, tag="rden")
                nc.vector.reciprocal(rden[:], den[:])
                aggs = blkp.tile([P, DIM], f32, tag="aggs")
                nc.vector.tensor_tensor(
                    out=aggs[:].rearrange("p (a b) -> p a b", b=DH),
                    in0=acc[:, 8:8 + DIM].rearrange("p (a b) -> p a b", b=DH),
                    in1=rden[:, :, None].to_broadcast([P, H, DH]),
                    op=mybir.AluOpType.mult,
                )
                psT2 = psmiscp.tile([P, P], f32, tag="T")
                nc.tensor.transpose(out=psT2[:], in_=aggs[:], identity=ident_sb[:])
                aggT = blkp.tile([P, P], f32r, tag="aggT")
                nc.scalar.copy(aggT[:], psT2[:])
                pso = psmiscp.tile([P, DIM], f32, tag="T")
                nc.tensor.matmul(pso[:], lhsT=aggT[:], rhs=Wo_sb[:],
                                 start=True, stop=False)
                nc.tensor.matmul(pso[:], lhsT=xT8c_sb[:, t * P:(t + 1) * P],
                                 rhs=J8_sb[:], start=False, stop=True)
                hrelu = blkp.tile([P, DIM], f32, tag="hrelu")
                nc.scalar.activation(hrelu[:], pso[:],
                                     mybir.ActivationFunctionType.Relu)
                nc.tensor.matmul(psS[:], lhsT=ones_sb[:], rhs=hrelu[:],
                                 start=(t == 0), stop=(t == TS - 1))
                if dbg and t == DBG_T:
                    tap("d_acc", acc[:])
                    tap("d_aggs", aggs[:])
                    tap("d_pso", pso[:])
                    tap("d_hrelu", hrelu[:])

            chunks = {}

            def load_chunk(ci):
                g0 = ci * GCHUNK
                gn = min(C, g0 + GCHUNK) - g0
                st = chunkp.tile([12, GCHUNK * P], f32r, tag="srcT")
                ohc = chunkp.tile([P, GCHUNK, P], sdt, tag="ohc")
                nc.sync.dma_start(out=st[:, :gn * P],
                                  in_=srcfacT_in[:, g0 * P:(g0 + gn) * P])
                nc.sync.dma_start(out=ohc[:, :gn, :], in_=ohmat_in[:, g0:g0 + gn, :])
                chunks[ci] = (st, ohc)

            acc = None
            for g in range(0, C, 2):
                ci, cb = g // GCHUNK, g % GCHUNK
                if cb == 0:
                    load_chunk(ci)
                st, ohc = chunks[ci]

                psm = psmainp.tile([P, 1024], f32, tag="main")
                psmv = psm[:].rearrange("p (c x) -> p c x", x=512)
                for q in range(2):
                    nc.tensor.matmul(psmv[:, q, 0:320],
                                     lhsT=st[:, (cb + q) * P:(cb + q + 1) * P],
                                     rhs=rhs12_sb[:], start=True, stop=True)

                gx = blkp.tile([P, 2, 64], f32, tag="gx")
                nc.vector.tensor_tensor(
                    out=gx[:].rearrange("p c (a b) -> p c a b", b=8),
                    in0=psmv[:, :, 256:320].rearrange("p c (a b) -> p c a b", b=8),
                    in1=dc[:, g:g + 2, None, :].to_broadcast([P, 2, 8, 8]),
                    op=mybir.AluOpType.mult,
                )
                lg = blkp.tile([P, 2, H], f32, tag="lg")
                nc.vector.tensor_reduce(
                    out=lg[:], in_=gx[:].rearrange("p c (a b) -> p c a b", b=8),
                    axis=mybir.AxisListType.X, op=mybir.AluOpType.add,
                )
                rhswm = blkp.tile([P, 2, 8 + DIM], sdt, tag="rhswm")
                nc.scalar.activation(rhswm[:, :, 0:8], lg[:],
                                     mybir.ActivationFunctionType.Exp)
                nc.vector.tensor_tensor(
                    out=rhswm[:, :, 8:8 + DIM].rearrange("p c (a b) -> p c a b", b=DH),
                    in0=psmv[:, :, 128:256].rearrange("p c (a b) -> p c a b", b=DH),
                    in1=rhswm[:, :, 0:8, None].to_broadcast([P, 2, H, DH]),
                    op=mybir.AluOpType.mult,
                )
                for q in range(2):
                    t, b, nb = blk2tile[g + q]
                    if b == 0:
                        acc = psaccp.tile([P, 8 + DIM], f32, tag="acc")
                    nc.tensor.matmul(acc[:], lhsT=ohc[:, cb + q, :],
                                     rhs=rhswm[:, q, :],
                                     start=(b == 0), stop=(b == nb - 1))
                    if dbg and g + q == DBG_G:
                        tap("d_dst8", dc[:, g + q, :])
                        tap("d_psm", psmv[:, q, 0:320])
                        tap("d_gx", gx[:, q, :])
                        tap("d_lg", lg[:, q, :])
                        tap("d_rhswm", rhswm[:, q, :])
                        tap("d_oh", ohc[:, cb + q, :])
                    if b == nb - 1:
                        _epilogue(t, acc)

            Scopy = constp.tile([1, DIM], f32, tag="Scopy")
            nc.vector.tensor_copy(Scopy[:], psS[:])
            nc.sync.dma_start(out=S_out, in_=Scopy[:])

    nc.compile()
    return nc


# ---------------------------------------------------------------- entry point
def kernel(**inputs):
    pos = np.asarray(inputs["pos"], np.float32)
    edge_attr = np.asarray(inputs["edge_attr"], np.float32)
    emb = np.asarray(inputs["emb"], np.float32)
    Wq = np.asarray(inputs["Wq"], np.float32)
    Wk = np.asarray(inputs["Wk"], np.float32)
    Wv = np.asarray(inputs["Wv"], np.float32)
    We = np.asarray(inputs["We"], np.float32)
    Wo = np.asarray(inputs["Wo"], np.float32)
    lin_w = np.asarray(inputs["lin_w"], np.float32)
    lin_b = np.asarray(inputs["lin_b"], np.float32)
    z = inputs["z"]
    edge_index = inputs["edge_index"]

    shared, percore, meta = _host_prep(pos, edge_attr, emb, Wq, Wk, Wv, We, z, edge_index)
    N = meta["N"]

    key = (tuple(meta["counts"]), meta["C"], meta["TS"], meta["npad"])
    nc = _PROG_CACHE.get(key)
    if nc is None:
        nc = _build_program(meta["counts"], meta["C"], meta["TS"], meta["npad"])
        _PROG_CACHE[key] = nc

    in_maps = []
    for j in range(N_CORES):
        m = {
            "rhs12": shared["rhs12"],
            "J8": shared["J8"],
            "ident": shared["ident"],
            "ones": shared["ones"],
            "Wo_in": Wo,
            "srcfacT": percore["srcfacT"][j],
            "dstfac": percore["dstfac"][j],
            "ohmat": percore["ohmat"][j],
            "xT8c": percore["xT8c"][j],
        }
        in_maps.append(m)

    from concourse.bass_utils import run_bass_kernel_spmd
    res = run_bass_kernel_spmd(nc, in_maps, core_ids=list(range(N_CORES)),
                               trace=PROFILE, trace_cores=TRACE_CORES)
    global LAST_RESULT
    LAST_RESULT = res
    S = np.zeros(DIM, np.float64)
    for r in res.results:
        S += r["S_out"][0].astype(np.float64)
    y = (S.astype(np.float32) @ lin_w) / np.float32(N) + lin_b
    return y.reshape(1, 1).astype(np.float32)


# revision 27
# speedup vs baseline: 2.2369x; 1.0971x over previous
"""EquivariantGNN message-passing kernel for Trainium2 (8 NeuronCores, SPMD).

Math (matches the reference):
  x   = [pos | onehot(z)] @ [[I3,0],[0,emb]]          (rank-8 node features)
  q/k/v = x @ W* = x8 @ W*8        with W*8 = [[W*[:3]],[emb @ W*[3:]]]  (8x128)
  ke  = k[src] + ea@We = [ea | x8[src]] @ [[We],[Wk8]]
  logits[e,h] = 0.25 * q[dst]. ke[e]  =  sum_i x8[dst][i] * G[e, h*8+i]
      where G = [ea | x8[src]] @ Bcat   (Bcat[j, h*8+i] = 0.25 * Wq8[i,hd].Wke12[j,hd])
  w = exp(logits)  (no max subtraction needed; logits are O(10))
  den[n,h] = sum_{dst(e)=n} w ;  agg[n] = (sum w*ve) / (den+1e-9)
  out = agg @ Wo + x ; S = sum_n relu(out) ; answer = (S @ lin_w)/N + lin_b

Device strategy per core: edges sorted by dst, 128-edge blocks each fully inside
one 128-node tile.  Per block: indirect-DMA gather of x8[src] (32B rows) into a
[128,12] tile (with ea), PE transpose -> lhsT12, one f32r matmul -> ke|ve|G,
DVE logits + exp, onehot(localdst) matmul scatter-accumulate of [w|w*ve] into a
per-tile PSUM accumulator.  Tile epilogue normalizes, applies Wo + residual,
relu, and accumulates the node-sum S via a ones-matmul.
"""

import math
import os
import sys

import numpy as np

for _p in ("/opt/trn_rl_repo", "/root/.axon_site/_ro/trn_rl_repo"):
    if os.path.isdir(_p) and _p not in sys.path:
        sys.path.insert(0, _p)

P = 128
DIM = 128
H, DH = 8, 16
DE = 4
N_CORES = 8
GCHUNK = 64  # blocks per gather chunk

# test-harness knobs (the grading harness just calls kernel() with defaults)
PROFILE = False
TRACE_CORES = None
LAST_RESULT = None  # BassKernelResults of the last run (for profiling)
_PROG_CACHE = {}


# ---------------------------------------------------------------- host prep
def _host_prep(pos, edge_attr, emb, Wq, Wk, Wv, We, z, edge_index):
    f32 = np.float32
    N = pos.shape[0]
    NT = emb.shape[0]
    ntiles = (N + P - 1) // P
    npad = ntiles * P

    z = np.asarray(z).astype(np.int64)
    src = np.asarray(edge_index[0]).astype(np.int64)
    dst = np.asarray(edge_index[1]).astype(np.int64)
    E = src.shape[0]

    onehot = np.zeros((N, NT), f32)
    onehot[np.arange(N), z] = 1.0
    x8 = np.concatenate([np.asarray(pos, f32), onehot], axis=1)  # [N, 8]
    x8p = np.zeros((npad, 8), f32)
    x8p[:N] = x8

    # rank-8 weight factors
    Wq8 = np.vstack([Wq[:3], emb @ Wq[3:]]).astype(f32)  # [8,128]
    Wk8 = np.vstack([Wk[:3], emb @ Wk[3:]]).astype(f32)
    Wv8 = np.vstack([Wv[:3], emb @ Wv[3:]]).astype(f32)
    Wke12 = np.vstack([Wk8, We]).astype(f32)  # [12,128]: rows = [x8src(8); ea(4)]

    # bilinear logits factors: logits[e,h] = sum_i x8dst[i] * G[e,h*8+i],
    # G[e, h*8+i] = sum_j lhs12[j,e] * Bc[j, h*8+i],  lhs12 rows = [ea(4); x8src(8)]
    Bc = np.zeros((12, 64), f32)
    for h in range(H):
        Bh = Wq8[:, h * DH:(h + 1) * DH] @ Wke12[:, h * DH:(h + 1) * DH].T  # [8,12]
        Bc[0:4, h * 8:(h + 1) * 8] = 0.25 * Bh[:, 8:12].T  # ea rows
        Bc[4:12, h * 8:(h + 1) * 8] = 0.25 * Bh[:, 0:8].T  # src rows
    rhs12 = np.zeros((12, 320), f32)
    rhs12[0:4, 0:128] = We
    rhs12[0:4, 128:256] = We
    rhs12[4:12, 0:128] = Wk8
    rhs12[4:12, 128:256] = Wv8
    rhs12[:, 256:320] = Bc

    J8 = np.zeros((8, DIM), f32)  # x = x8 @ J8
    J8[0:3, 0:3] = np.eye(3, dtype=f32)
    J8[3:8, 3:DIM] = emb

    # ---- sort edges by dst, split into per-node-tile runs
    perm = np.argsort(dst, kind="stable")
    src_s, dst_s = src[perm], dst[perm]
    ea_s = np.asarray(edge_attr, f32)[perm]
    tile_of_edge = dst_s // P
    starts = np.searchsorted(tile_of_edge, np.arange(ntiles))
    ends = np.searchsorted(tile_of_edge, np.arange(ntiles) + 1)
    ecnt = ends - starts
    nb = np.maximum(1, (ecnt + P - 1) // P)  # blocks per real tile

    # ---- uniform schedule across cores: pad tile list to multiple of 8,
    # sort by block count desc, deal groups of 8 (one tile per core),
    # pad each group to the group max -> identical counts on every core.
    ntiles_tot = ((ntiles + N_CORES - 1) // N_CORES) * N_CORES
    nb_all = np.concatenate([nb, np.ones(ntiles_tot - ntiles, np.int64)])
    order = np.argsort(-nb_all, kind="stable")
    TS = ntiles_tot // N_CORES  # tiles per core
    counts = [int(nb_all[order[8 * k]]) for k in range(TS)]  # group max (sorted desc)
    if sum(counts) % 2:
        counts[-1] += 1  # keep global block count even for pair processing
    C = int(sum(counts))

    import ml_dtypes

    srcfac = np.zeros((N_CORES, C, P, 12), f32)  # [ea(4) | x8[src](8)]
    dstfac = np.zeros((N_CORES, C, P, 8), f32)  # x8[dst]
    ohmat = np.zeros((N_CORES, C, P, P), ml_dtypes.bfloat16)  # onehot(localdst)
    xT8c = np.zeros((N_CORES, 8, TS * P), f32)

    offs = np.concatenate([[0], np.cumsum(counts)])
    for k in range(TS):
        for j in range(N_CORES):
            t = int(order[8 * k + j])
            if t >= ntiles:
                continue  # dummy tile: all-dummy blocks, zero xT8c
            xT8c[j, :, k * P:(k + 1) * P] = x8p[t * P:(t + 1) * P].T
            e0, e1 = int(starts[t]), int(ends[t])
            ne = e1 - e0
            if ne == 0:
                continue
            c0 = int(offs[k])
            flat = np.arange(ne)
            cc = c0 + flat // P
            pp = flat % P
            srcfac[j, cc, pp, 0:DE] = ea_s[e0:e1]
            srcfac[j, cc, pp, DE:12] = x8[src_s[e0:e1]]
            dstfac[j, cc, pp, :] = x8[dst_s[e0:e1]]
            ohmat[j, cc, pp, dst_s[e0:e1] - t * P] = 1.0

    ident = np.eye(P, dtype=f32)
    ones = np.ones((P, 1), f32)

    # device layouts
    srcfacT = np.ascontiguousarray(
        srcfac.transpose(0, 3, 1, 2)).reshape(N_CORES, 12, C * P).astype(
            ml_dtypes.bfloat16)
    dstfacd = np.ascontiguousarray(dstfac.transpose(0, 2, 1, 3))  # [j, P, C, 8]
    ohmatd = np.ascontiguousarray(ohmat.transpose(0, 2, 1, 3))  # [j, P, C, P]

    shared = dict(rhs12=rhs12.astype(ml_dtypes.bfloat16), J8=J8, ident=ident,
                  ones=ones)
    percore = dict(srcfacT=srcfacT, dstfac=dstfacd, ohmat=ohmatd, xT8c=xT8c)
    meta = dict(counts=counts, C=C, TS=TS, npad=npad, N=N, E=E)
    return shared, percore, meta


# ---------------------------------------------------------------- device code
DBG_T = 0  # tile index to tap when dbg=True
DBG_G = 0  # global block index to tap when dbg=True


def _build_program(counts, C, TS, npad, use_f32r=True, scatter_bf16=True, dbg=False):
    import concourse.bacc as bacc
    import concourse.bass as bass
    import concourse.tile as tile
    from concourse import mybir
    from concourse._compat import with_exitstack  # noqa: F401

    f32 = mybir.dt.float32
    f32r = mybir.dt.float32r if use_f32r else mybir.dt.float32
    bf16 = mybir.dt.bfloat16
    i32 = mybir.dt.int32
    sdt = bf16 if scatter_bf16 else f32

    nc = bacc.Bacc("TRN2", target_bir_lowering=False, debug=False,
                   enable_asserts=False, num_devices=N_CORES)

    srcfacT_in = nc.dram_tensor("srcfacT", [12, C * P], bf16, kind="ExternalInput").ap()
    dstfac_in = nc.dram_tensor("dstfac", [P, C, 8], f32, kind="ExternalInput").ap()
    ohmat_in = nc.dram_tensor("ohmat", [P, C, P], sdt, kind="ExternalInput").ap()
    xT8c_in = nc.dram_tensor("xT8c", [8, TS * P], f32r, kind="ExternalInput").ap()
    rhs12_in = nc.dram_tensor("rhs12", [12, 320], bf16, kind="ExternalInput").ap()
    J8_in = nc.dram_tensor("J8", [8, DIM], f32r, kind="ExternalInput").ap()
    Wo_in = nc.dram_tensor("Wo_in", [DIM, DIM], f32r, kind="ExternalInput").ap()
    ident_in = nc.dram_tensor("ident", [P, P], f32, kind="ExternalInput").ap()
    ones_in = nc.dram_tensor("ones", [P, 1], f32, kind="ExternalInput").ap()
    S_out = nc.dram_tensor("S_out", [1, DIM], f32, kind="ExternalOutput").ap()
    dbg_outs = {}
    if dbg:
        for nm, shp in [("d_dst8", [P, 8]), ("d_psm", [P, 320]),
                        ("d_gx", [P, 64]), ("d_lg", [P, H]),
                        ("d_rhswm", [P, 8 + DIM]), ("d_oh", [P, P]),
                        ("d_acc", [P, 8 + DIM]), ("d_aggs", [P, DIM]),
                        ("d_pso", [P, DIM]), ("d_hrelu", [P, DIM])]:
            dbg_outs[nm] = nc.dram_tensor(nm, shp, f32, kind="ExternalOutput").ap()

    with tile.TileContext(nc) as tc:
        with (
            tc.tile_pool(name="const", bufs=1) as constp,
            tc.tile_pool(name="chunks", bufs=2) as chunkp,
            tc.tile_pool(name="blk", bufs=3) as blkp,
            tc.tile_pool(name="psmain", bufs=2, space="PSUM") as psmainp,
            tc.tile_pool(name="psmisc", bufs=1, space="PSUM") as psmiscp,
            tc.tile_pool(name="psacc", bufs=2, space="PSUM") as psaccp,
            tc.tile_pool(name="psS", bufs=1, space="PSUM") as psSp,
        ):
            dc = constp.tile_from(dstfac_in)
            xT8c_sb = constp.tile_from(xT8c_in)
            rhs12_sb = constp.tile_from(rhs12_in)
            J8_sb = constp.tile_from(J8_in)
            Wo_sb = constp.tile_from(Wo_in)
            ident_sb = constp.tile_from(ident_in)
            ones_sb = constp.tile_from(ones_in)

            psS = psSp.tile([1, DIM], f32, tag="S")

            def tap(name, ap):
                if not dbg or name not in dbg_outs:
                    return
                tmp = constp.tile(list(ap.shape), f32, tag="tap_" + name)
                nc.vector.tensor_copy(tmp[:], ap)
                nc.sync.dma_start(out=dbg_outs[name], in_=tmp[:])

            # block -> (tile, b, nb) map for the flat pair loop
            blk2tile = []
            for t in range(TS):
                for b in range(counts[t]):
                    blk2tile.append((t, b, counts[t]))

            def _epilogue(t, acc):
                den = blkp.tile([P, H], f32, tag="den")
                nc.vector.tensor_scalar_add(den[:], acc[:, 0:8], 1e-9)
                rden = blkp.tile([P, H], f32, tag="rden")
                nc.vector.reciprocal(rden[:], den[:])
                aggs = blkp.tile([P, DIM], f32, tag="aggs")
                nc.vector.tensor_tensor(
                    out=aggs[:].rearrange("p (a b) -> p a b", b=DH),
                    in0=acc[:, 8:8 + DIM].rearrange("p (a b) -> p a b", b=DH),
                    in1=rden[:, :, None].to_broadcast([P, H, DH]),
                    op=mybir.AluOpType.mult,
                )
                psT2 = psmiscp.tile([P, P], f32, tag="T")
                nc.tensor.transpose(out=psT2[:], in_=aggs[:], identity=ident_sb[:])
                aggT = blkp.tile([P, P], f32r, tag="aggT")
                nc.scalar.copy(aggT[:], psT2[:])
                pso = psmiscp.tile([P, DIM], f32, tag="T")
                nc.tensor.matmul(pso[:], lhsT=aggT[:], rhs=Wo_sb[:],
                                 start=True, stop=False)
                nc.tensor.matmul(pso[:], lhsT=xT8c_sb[:, t * P:(t + 1) * P],
                                 rhs=J8_sb[:], start=False, stop=True)
                hrelu = blkp.tile([P, DIM], f32, tag="hrelu")
                nc.scalar.activation(hrelu[:], pso[:],
                                     mybir.ActivationFunctionType.Relu)
                nc.tensor.matmul(psS[:], lhsT=ones_sb[:], rhs=hrelu[:],
                                 start=(t == 0), stop=(t == TS - 1))
                if dbg and t == DBG_T:
                    tap("d_acc", acc[:])
                    tap("d_aggs", aggs[:])
                    tap("d_pso", pso[:])
                    tap("d_hrelu", hrelu[:])

            chunks = {}

            def load_chunk(ci):
                g0 = ci * GCHUNK
                gn = min(C, g0 + GCHUNK) - g0
                st = chunkp.tile([12, GCHUNK * P], bf16, tag="srcT")
                ohc = chunkp.tile([P, GCHUNK, P], sdt, tag="ohc")
                nc.sync.dma_start(out=st[:, :gn * P],
                                  in_=srcfacT_in[:, g0 * P:(g0 + gn) * P])
                nc.sync.dma_start(out=ohc[:, :gn, :], in_=ohmat_in[:, g0:g0 + gn, :])
                chunks[ci] = (st, ohc)

            acc = None
            for g in range(0, C, 2):
                ci, cb = g // GCHUNK, g % GCHUNK
                if cb == 0:
                    load_chunk(ci)
                st, ohc = chunks[ci]

                psm = psmainp.tile([P, 1024], f32, tag="main")
                psmv = psm[:].rearrange("p (c x) -> p c x", x=512)
                for q in range(2):
                    nc.tensor.matmul(psmv[:, q, 0:320],
                                     lhsT=st[:, (cb + q) * P:(cb + q + 1) * P],
                                     rhs=rhs12_sb[:], start=True, stop=True)

                gx = blkp.tile([P, 2, 64], f32, tag="gx")
                nc.vector.tensor_tensor(
                    out=gx[:].rearrange("p c (a b) -> p c a b", b=8),
                    in0=psmv[:, :, 256:320].rearrange("p c (a b) -> p c a b", b=8),
                    in1=dc[:, g:g + 2, None, :].to_broadcast([P, 2, 8, 8]),
                    op=mybir.AluOpType.mult,
                )
                lg = blkp.tile([P, 2, H], f32, tag="lg")
                nc.vector.tensor_reduce(
                    out=lg[:], in_=gx[:].rearrange("p c (a b) -> p c a b", b=8),
                    axis=mybir.AxisListType.X, op=mybir.AluOpType.add,
                )
                rhswm = blkp.tile([P, 2, 8 + DIM], sdt, tag="rhswm")
                nc.scalar.activation(rhswm[:, :, 0:8], lg[:],
                                     mybir.ActivationFunctionType.Exp)
                nc.vector.tensor_tensor(
                    out=rhswm[:, :, 8:8 + DIM].rearrange("p c (a b) -> p c a b", b=DH),
                    in0=psmv[:, :, 128:256].rearrange("p c (a b) -> p c a b", b=DH),
                    in1=rhswm[:, :, 0:8, None].to_broadcast([P, 2, H, DH]),
                    op=mybir.AluOpType.mult,
                )
                for q in range(2):
                    t, b, nb = blk2tile[g + q]
                    if b == 0:
                        acc = psaccp.tile([P, 8 + DIM], f32, tag="acc")
                    nc.tensor.matmul(acc[:], lhsT=ohc[:, cb + q, :],
                                     rhs=rhswm[:, q, :],
                                     start=(b == 0), stop=(b == nb - 1))
                    if dbg and g + q == DBG_G:
                        tap("d_dst8", dc[:, g + q, :])
                        tap("d_psm", psmv[:, q, 0:320])
                        tap("d_gx", gx[:, q, :])
                        tap("d_lg", lg[:, q, :])
                        tap("d_rhswm", rhswm[:, q, :])
                        tap("d_oh", ohc[:, cb + q, :])
                    if b == nb - 1:
                        _epilogue(t, acc)

            Scopy = constp.tile([1, DIM], f32, tag="Scopy")
            nc.vector.tensor_copy(Scopy[:], psS[:])
            nc.sync.dma_start(out=S_out, in_=Scopy[:])

    nc.compile()
    return nc


# ---------------------------------------------------------------- entry point
def kernel(**inputs):
    pos = np.asarray(inputs["pos"], np.float32)
    edge_attr = np.asarray(inputs["edge_attr"], np.float32)
    emb = np.asarray(inputs["emb"], np.float32)
    Wq = np.asarray(inputs["Wq"], np.float32)
    Wk = np.asarray(inputs["Wk"], np.float32)
    Wv = np.asarray(inputs["Wv"], np.float32)
    We = np.asarray(inputs["We"], np.float32)
    Wo = np.asarray(inputs["Wo"], np.float32)
    lin_w = np.asarray(inputs["lin_w"], np.float32)
    lin_b = np.asarray(inputs["lin_b"], np.float32)
    z = inputs["z"]
    edge_index = inputs["edge_index"]

    shared, percore, meta = _host_prep(pos, edge_attr, emb, Wq, Wk, Wv, We, z, edge_index)
    N = meta["N"]

    key = (tuple(meta["counts"]), meta["C"], meta["TS"], meta["npad"])
    nc = _PROG_CACHE.get(key)
    if nc is None:
        nc = _build_program(meta["counts"], meta["C"], meta["TS"], meta["npad"])
        _PROG_CACHE[key] = nc

    in_maps = []
    for j in range(N_CORES):
        m = {
            "rhs12": shared["rhs12"],
            "J8": shared["J8"],
            "ident": shared["ident"],
            "ones": shared["ones"],
            "Wo_in": Wo,
            "srcfacT": percore["srcfacT"][j],
            "dstfac": percore["dstfac"][j],
            "ohmat": percore["ohmat"][j],
            "xT8c": percore["xT8c"][j],
        }
        in_maps.append(m)

    from concourse.bass_utils import run_bass_kernel_spmd
    res = run_bass_kernel_spmd(nc, in_maps, core_ids=list(range(N_CORES)),
                               trace=PROFILE, trace_cores=TRACE_CORES)
    global LAST_RESULT
    LAST_RESULT = res
    S = np.zeros(DIM, np.float64)
    for r in res.results:
        S += r["S_out"][0].astype(np.float64)
    y = (S.astype(np.float32) @ lin_w) / np.float32(N) + lin_b
    return y.reshape(1, 1).astype(np.float32)


# revision 38
# speedup vs baseline: 3.4619x; 1.5477x over previous
"""EquivariantGNN message-passing kernel for Trainium2 (8 NeuronCores, SPMD).

Math (matches the reference):
  x   = [pos | onehot(z)] @ [[I3,0],[0,emb]]          (rank-8 node features)
  q/k/v = x @ W* = x8 @ W*8        with W*8 = [[W*[:3]],[emb @ W*[3:]]]  (8x128)
  ke  = k[src] + ea@We = [ea | x8[src]] @ [[We],[Wk8]]
  logits[e,h] = 0.25 * q[dst]. ke[e]  =  sum_i x8[dst][i] * G[e, h*8+i]
      where G = [ea | x8[src]] @ Bcat   (Bcat[j, h*8+i] = 0.25 * Wq8[i,hd].Wke12[j,hd])
  w = exp(logits)  (no max subtraction needed; logits are O(10))
  den[n,h] = sum_{dst(e)=n} w ;  agg[n] = (sum w*ve) / (den+1e-9)
  out = agg @ Wo + x ; S = sum_n relu(out) ; answer = (S @ lin_w)/N + lin_b

Device strategy per core: edges sorted by dst, 128-edge blocks each fully inside
one 128-node tile.  Per block: indirect-DMA gather of x8[src] (32B rows) into a
[128,12] tile (with ea), PE transpose -> lhsT12, one f32r matmul -> ke|ve|G,
DVE logits + exp, onehot(localdst) matmul scatter-accumulate of [w|w*ve] into a
per-tile PSUM accumulator.  Tile epilogue normalizes, applies Wo + residual,
relu, and accumulates the node-sum S via a ones-matmul.
"""

import math
import os
import sys

import numpy as np

for _p in ("/opt/trn_rl_repo", "/root/.axon_site/_ro/trn_rl_repo"):
    if os.path.isdir(_p) and _p not in sys.path:
        sys.path.insert(0, _p)

P = 128
DIM = 128
H, DH = 8, 16
DE = 4
N_CORES = 8
GCHUNK = 64  # blocks per gather chunk

# test-harness knobs (the grading harness just calls kernel() with defaults)
PROFILE = False
TRACE_CORES = None
LAST_RESULT = None  # BassKernelResults of the last run (for profiling)
_PROG_CACHE = {}


# ---------------------------------------------------------------- host prep
def _host_prep(pos, edge_attr, emb, Wq, Wk, Wv, We, z, edge_index):
    f32 = np.float32
    N = pos.shape[0]
    NT = emb.shape[0]
    ntiles = (N + P - 1) // P
    npad = ntiles * P

    z = np.asarray(z).astype(np.int64)
    src = np.asarray(edge_index[0]).astype(np.int64)
    dst = np.asarray(edge_index[1]).astype(np.int64)
    E = src.shape[0]

    onehot = np.zeros((N, NT), f32)
    onehot[np.arange(N), z] = 1.0
    x8 = np.concatenate([np.asarray(pos, f32), onehot], axis=1)  # [N, 8]
    x8p = np.zeros((npad, 8), f32)
    x8p[:N] = x8

    # rank-8 weight factors
    Wq8 = np.vstack([Wq[:3], emb @ Wq[3:]]).astype(f32)  # [8,128]
    Wk8 = np.vstack([Wk[:3], emb @ Wk[3:]]).astype(f32)
    Wv8 = np.vstack([Wv[:3], emb @ Wv[3:]]).astype(f32)
    Wke12 = np.vstack([Wk8, We]).astype(f32)  # [12,128]: rows = [x8src(8); ea(4)]

    # bilinear logits factors: logits[e,h] = sum_i x8dst[i] * G[e,h*8+i],
    # G[e, h*8+i] = sum_j lhs12[j,e] * Bc[j, h*8+i],  lhs12 rows = [ea(4); x8src(8)]
    Bc = np.zeros((12, 64), f32)
    for h in range(H):
        Bh = Wq8[:, h * DH:(h + 1) * DH] @ Wke12[:, h * DH:(h + 1) * DH].T  # [8,12]
        Bc[0:4, h * 8:(h + 1) * 8] = 0.25 * Bh[:, 8:12].T  # ea rows
        Bc[4:12, h * 8:(h + 1) * 8] = 0.25 * Bh[:, 0:8].T  # src rows
    # moving weights: [ve (128) | G (64)] — ke is folded into Bc, never materialized
    rhs12 = np.zeros((12, 192), f32)
    rhs12[0:4, 0:128] = We
    rhs12[4:12, 0:128] = Wv8
    rhs12[:, 128:192] = Bc

    J8 = np.zeros((8, DIM), f32)  # x = x8 @ J8
    J8[0:3, 0:3] = np.eye(3, dtype=f32)
    J8[3:8, 3:DIM] = emb

    # ---- sort edges by dst, split into per-node-tile runs
    perm = np.argsort(dst, kind="stable")
    src_s, dst_s = src[perm], dst[perm]
    ea_s = np.asarray(edge_attr, f32)[perm]
    tile_of_edge = dst_s // P
    starts = np.searchsorted(tile_of_edge, np.arange(ntiles))
    ends = np.searchsorted(tile_of_edge, np.arange(ntiles) + 1)
    ecnt = ends - starts
    nb = np.maximum(1, (ecnt + P - 1) // P)  # blocks per real tile

    # ---- uniform schedule across cores: pad tile list to multiple of 8,
    # sort by block count desc, deal groups of 8 (one tile per core),
    # pad each group to the group max -> identical counts on every core.
    ntiles_tot = ((ntiles + N_CORES - 1) // N_CORES) * N_CORES
    nb_all = np.concatenate([nb, np.ones(ntiles_tot - ntiles, np.int64)])
    order = np.argsort(-nb_all, kind="stable")
    TS = ntiles_tot // N_CORES  # tiles per core
    counts = [int(nb_all[order[8 * k]]) for k in range(TS)]  # group max (sorted desc)
    counts[-1] += (-sum(counts)) % 4  # block count multiple of 4 (quad processing)
    C = int(sum(counts))

    import ml_dtypes

    srcfac = np.zeros((N_CORES, C, P, 12), f32)  # [ea(4) | x8[src](8)]
    dstfac = np.zeros((N_CORES, C, P, 8), f32)  # x8[dst]
    ohmat = np.zeros((N_CORES, C, P, P), ml_dtypes.bfloat16)  # onehot(localdst)
    xT8c = np.zeros((N_CORES, 8, TS * P), f32)

    offs = np.concatenate([[0], np.cumsum(counts)])
    for k in range(TS):
        for j in range(N_CORES):
            t = int(order[8 * k + j])
            if t >= ntiles:
                continue  # dummy tile: all-dummy blocks, zero xT8c
            xT8c[j, :, k * P:(k + 1) * P] = x8p[t * P:(t + 1) * P].T
            e0, e1 = int(starts[t]), int(ends[t])
            ne = e1 - e0
            if ne == 0:
                continue
            c0 = int(offs[k])
            flat = np.arange(ne)
            cc = c0 + flat // P
            pp = flat % P
            srcfac[j, cc, pp, 0:DE] = ea_s[e0:e1]
            srcfac[j, cc, pp, DE:12] = x8[src_s[e0:e1]]
            dstfac[j, cc, pp, :] = x8[dst_s[e0:e1]]
            ohmat[j, cc, pp, dst_s[e0:e1] - t * P] = 1.0

    ident = np.eye(P, dtype=f32)
    ones = np.ones((P, 1), f32)

    # device layouts
    srcfacT = np.ascontiguousarray(
        srcfac.transpose(0, 3, 1, 2)).reshape(N_CORES, 12, C * P).astype(
            ml_dtypes.bfloat16)
    dstfacd = np.ascontiguousarray(dstfac.transpose(0, 2, 1, 3))  # [j, P, C, 8]
    ohmatd = np.ascontiguousarray(ohmat.transpose(0, 2, 1, 3))  # [j, P, C, P]

    shared = dict(rhs12=rhs12.astype(ml_dtypes.bfloat16),
                  J8=J8.astype(ml_dtypes.bfloat16), ident=ident,
                  ones=ones.astype(ml_dtypes.bfloat16))
    percore = dict(srcfacT=srcfacT, dstfac=dstfacd, ohmat=ohmatd,
                   xT8c=xT8c.astype(ml_dtypes.bfloat16))
    meta = dict(counts=counts, C=C, TS=TS, npad=npad, N=N, E=E)
    return shared, percore, meta


# ---------------------------------------------------------------- device code
DBG_T = 0  # tile index to tap when dbg=True
DBG_G = 0  # global block index to tap when dbg=True


def _build_program(counts, C, TS, npad, use_f32r=True, scatter_bf16=True, dbg=False):
    import concourse.bacc as bacc
    import concourse.bass as bass
    import concourse.tile as tile
    from concourse import mybir
    from concourse._compat import with_exitstack  # noqa: F401

    f32 = mybir.dt.float32
    f32r = mybir.dt.float32r if use_f32r else mybir.dt.float32
    bf16 = mybir.dt.bfloat16
    i32 = mybir.dt.int32
    sdt = bf16 if scatter_bf16 else f32

    nc = bacc.Bacc("TRN2", target_bir_lowering=False, debug=False,
                   enable_asserts=False, num_devices=N_CORES)

    srcfacT_in = nc.dram_tensor("srcfacT", [12, C * P], bf16, kind="ExternalInput").ap()
    dstfac_in = nc.dram_tensor("dstfac", [P, C, 8], f32, kind="ExternalInput").ap()
    ohmat_in = nc.dram_tensor("ohmat", [P, C, P], sdt, kind="ExternalInput").ap()
    xT8c_in = nc.dram_tensor("xT8c", [8, TS * P], bf16, kind="ExternalInput").ap()
    rhs12_in = nc.dram_tensor("rhs12", [12, 192], bf16, kind="ExternalInput").ap()
    J8_in = nc.dram_tensor("J8", [8, DIM], bf16, kind="ExternalInput").ap()
    Wo_in = nc.dram_tensor("Wo_in", [DIM, DIM], bf16, kind="ExternalInput").ap()
    ident_in = nc.dram_tensor("ident", [P, P], f32, kind="ExternalInput").ap()
    ones_in = nc.dram_tensor("ones", [P, 1], bf16, kind="ExternalInput").ap()
    S_out = nc.dram_tensor("S_out", [1, DIM], f32, kind="ExternalOutput").ap()
    dbg_outs = {}
    if dbg:
        for nm, shp in [("d_dst8", [P, 8]), ("d_psm", [P, 192]),
                        ("d_gx", [P, 64]), ("d_lg", [P, H]),
                        ("d_rhswm", [P, 8 + DIM]), ("d_oh", [P, P]),
                        ("d_acc", [P, 8 + DIM]), ("d_aggs", [P, DIM]),
                        ("d_pso", [P, DIM]), ("d_hrelu", [P, DIM])]:
            dbg_outs[nm] = nc.dram_tensor(nm, shp, f32, kind="ExternalOutput").ap()

    with tile.TileContext(nc) as tc:
        with (
            tc.tile_pool(name="const", bufs=1) as constp,
            tc.tile_pool(name="chunks", bufs=2) as chunkp,
            tc.tile_pool(name="blk", bufs=4) as blkp,
            tc.tile_pool(name="psmain", bufs=2, space="PSUM") as psmainp,
            tc.tile_pool(name="psmisc", bufs=1, space="PSUM") as psmiscp,
            tc.tile_pool(name="psacc", bufs=2, space="PSUM") as psaccp,
            tc.tile_pool(name="psS", bufs=1, space="PSUM") as psSp,
        ):
            dc = constp.tile_from(dstfac_in)
            xT8c_sb = constp.tile_from(xT8c_in)
            rhs12_sb = constp.tile_from(rhs12_in)
            J8_sb = constp.tile_from(J8_in)
            Wo_sb = constp.tile_from(Wo_in)
            ident_sb = constp.tile_from(ident_in)
            ones_sb = constp.tile_from(ones_in)

            psS = psSp.tile([1, DIM], f32, tag="S")

            def tap(name, ap):
                if not dbg or name not in dbg_outs:
                    return
                tmp = constp.tile(list(ap.shape), f32, tag="tap_" + name)
                nc.vector.tensor_copy(tmp[:], ap)
                nc.sync.dma_start(out=dbg_outs[name], in_=tmp[:])

            # block -> (tile, b, nb) map for the flat pair loop
            blk2tile = []
            for t in range(TS):
                for b in range(counts[t]):
                    blk2tile.append((t, b, counts[t]))

            def _epilogue(t, acc):
                den = blkp.tile([P, H], f32, tag="den")
                nc.vector.tensor_scalar_add(den[:], acc[:, 0:8], 1e-9)
                rden = blkp.tile([P, H], f32, tag="rden")
                nc.vector.reciprocal(rden[:], den[:])
                aggs = blkp.tile([P, DIM], f32, tag="aggs")
                nc.vector.tensor_tensor(
                    out=aggs[:].rearrange("p (a b) -> p a b", b=DH),
                    in0=acc[:, 8:8 + DIM].rearrange("p (a b) -> p a b", b=DH),
                    in1=rden[:, :, None].to_broadcast([P, H, DH]),
                    op=mybir.AluOpType.mult,
                )
                psT2 = psmiscp.tile([P, P], f32, tag="T")
                nc.tensor.transpose(out=psT2[:], in_=aggs[:], identity=ident_sb[:])
                aggT = blkp.tile([P, P], bf16, tag="aggT")
                nc.scalar.copy(aggT[:], psT2[:])
                pso = psmiscp.tile([P, DIM], f32, tag="T")
                nc.tensor.matmul(pso[:], lhsT=aggT[:], rhs=Wo_sb[:],
                                 start=True, stop=False)
                nc.tensor.matmul(pso[:], lhsT=xT8c_sb[:, t * P:(t + 1) * P],
                                 rhs=J8_sb[:], start=False, stop=True)
                hrelu = blkp.tile([P, DIM], bf16, tag="hrelu")
                nc.scalar.activation(hrelu[:], pso[:],
                                     mybir.ActivationFunctionType.Relu)
                nc.tensor.matmul(psS[:], lhsT=ones_sb[:], rhs=hrelu[:],
                                 start=(t == 0), stop=(t == TS - 1))
                if dbg and t == DBG_T:
                    tap("d_acc", acc[:])
                    tap("d_aggs", aggs[:])
                    tap("d_pso", pso[:])
                    tap("d_hrelu", hrelu[:])

            chunks = {}

            def load_chunk(ci):
                g0 = ci * GCHUNK
                gn = min(C, g0 + GCHUNK) - g0
                st = chunkp.tile([12, GCHUNK * P], bf16, tag="srcT")
                ohc = chunkp.tile([P, GCHUNK, P], sdt, tag="ohc")
                nc.sync.dma_start(out=st[:, :gn * P],
                                  in_=srcfacT_in[:, g0 * P:(g0 + gn) * P])
                nc.sync.dma_start(out=ohc[:, :gn, :], in_=ohmat_in[:, g0:g0 + gn, :])
                chunks[ci] = (st, ohc)

            acc_state = [None]

            def flush_scatters(item):
                g0, rhswm, ohc, cb0 = item
                for q in range(4):
                    t, b, nb = blk2tile[g0 + q]
                    if b == 0:
                        acc_state[0] = psaccp.tile([P, 8 + DIM], f32, tag="acc",
                                                   name="acc")
                    acc = acc_state[0]
                    nc.tensor.matmul(acc[:], lhsT=ohc[:, cb0 + q, :],
                                     rhs=rhswm[:, q, :],
                                     start=(b == 0), stop=(b == nb - 1))
                    if dbg and g0 + q == DBG_G:
                        tap("d_oh", ohc[:, cb0 + q, :])
                    if b == nb - 1:
                        _epilogue(t, acc)

            pend = []
            for g in range(0, C, 4):
                ci, cb = g // GCHUNK, g % GCHUNK
                if cb == 0:
                    load_chunk(ci)
                st, ohc = chunks[ci]

                psm = psmainp.tile([P, 1024], f32, tag="main")
                psmv = psm[:].rearrange("p (c x) -> p c x", x=256)
                for q in range(4):
                    nc.tensor.matmul(psmv[:, q, 0:192],
                                     lhsT=st[:, (cb + q) * P:(cb + q + 1) * P],
                                     rhs=rhs12_sb[:], start=True, stop=True)

                gx = blkp.tile([P, 4, 64], f32, tag="gx")
                nc.vector.tensor_tensor(
                    out=gx[:].rearrange("p c (a b) -> p c a b", b=8),
                    in0=psmv[:, :, 128:192].rearrange("p c (a b) -> p c a b", b=8),
                    in1=dc[:, g:g + 4, None, :].to_broadcast([P, 4, 8, 8]),
                    op=mybir.AluOpType.mult,
                )
                lg = blkp.tile([P, 4, H], f32, tag="lg")
                nc.vector.tensor_reduce(
                    out=lg[:], in_=gx[:].rearrange("p c (a b) -> p c a b", b=8),
                    axis=mybir.AxisListType.X, op=mybir.AluOpType.add,
                )
                rhswm = blkp.tile([P, 4, 8 + DIM], sdt, tag="rhswm")
                nc.scalar.activation(rhswm[:, :, 0:8], lg[:],
                                     mybir.ActivationFunctionType.Exp)
                nc.vector.tensor_tensor(
                    out=rhswm[:, :, 8:8 + DIM].rearrange("p c (a b) -> p c a b", b=DH),
                    in0=psmv[:, :, 0:128].rearrange("p c (a b) -> p c a b", b=DH),
                    in1=rhswm[:, :, 0:8, None].to_broadcast([P, 4, H, DH]),
                    op=mybir.AluOpType.mult,
                )
                if dbg and DBG_G // 4 == g // 4:
                    q = DBG_G % 4
                    tap("d_dst8", dc[:, DBG_G, :])
                    tap("d_psm", psmv[:, q, 0:192])
                    tap("d_gx", gx[:, q, :])
                    tap("d_lg", lg[:, q, :])
                    tap("d_rhswm", rhswm[:, q, :])
                pend.append((g, rhswm, ohc, cb))
                if len(pend) > 1:
                    flush_scatters(pend.pop(0))
            while pend:
                flush_scatters(pend.pop(0))

            Scopy = constp.tile([1, DIM], f32, tag="Scopy")
            nc.vector.tensor_copy(Scopy[:], psS[:])
            nc.sync.dma_start(out=S_out, in_=Scopy[:])

    nc.compile()
    return nc


def _bf16(a):
    import ml_dtypes
    return np.asarray(a).astype(ml_dtypes.bfloat16)


# ---------------------------------------------------------------- entry point
def kernel(**inputs):
    pos = np.asarray(inputs["pos"], np.float32)
    edge_attr = np.asarray(inputs["edge_attr"], np.float32)
    emb = np.asarray(inputs["emb"], np.float32)
    Wq = np.asarray(inputs["Wq"], np.float32)
    Wk = np.asarray(inputs["Wk"], np.float32)
    Wv = np.asarray(inputs["Wv"], np.float32)
    We = np.asarray(inputs["We"], np.float32)
    Wo = np.asarray(inputs["Wo"], np.float32)
    lin_w = np.asarray(inputs["lin_w"], np.float32)
    lin_b = np.asarray(inputs["lin_b"], np.float32)
    z = inputs["z"]
    edge_index = inputs["edge_index"]

    shared, percore, meta = _host_prep(pos, edge_attr, emb, Wq, Wk, Wv, We, z, edge_index)
    N = meta["N"]

    key = (tuple(meta["counts"]), meta["C"], meta["TS"], meta["npad"])
    nc = _PROG_CACHE.get(key)
    if nc is None:
        nc = _build_program(meta["counts"], meta["C"], meta["TS"], meta["npad"])
        _PROG_CACHE[key] = nc

    in_maps = []
    for j in range(N_CORES):
        m = {
            "rhs12": shared["rhs12"],
            "J8": shared["J8"],
            "ident": shared["ident"],
            "ones": shared["ones"],
            "Wo_in": _bf16(Wo),
            "srcfacT": percore["srcfacT"][j],
            "dstfac": percore["dstfac"][j],
            "ohmat": percore["ohmat"][j],
            "xT8c": percore["xT8c"][j],
        }
        in_maps.append(m)

    from concourse.bass_utils import run_bass_kernel_spmd
    res = run_bass_kernel_spmd(nc, in_maps, core_ids=list(range(N_CORES)),
                               trace=PROFILE, trace_cores=TRACE_CORES)
    global LAST_RESULT
    LAST_RESULT = res
    S = np.zeros(DIM, np.float64)
    for r in res.results:
        S += r["S_out"][0].astype(np.float64)
    y = (S.astype(np.float32) @ lin_w) / np.float32(N) + lin_b
    return y.reshape(1, 1).astype(np.float32)
